# revision 43
# baseline (speedup 1.0000x reference)
"""AutoBoxGraphAttention Trainium2 kernel (v2 - PE-warm restructure).

Data-parallel over batch: core b handles image b (B=8, one per NeuronCore).
The only cross-core communication is a 4KB AllReduce of BatchNorm partials.

Layout worlds:
  conv world      : channel on partitions, spatial flat on free dim
  attention world : spatial row (d) / col (W') on partitions, per (head, i)
Transitions are free for q,k,v: the qkv 1x1 conv is computed with the
ACTIVATION as the stationary matmul operand (x^T column-pair chunks for
q,k; x row-pair chunks for v), so outputs land directly in attention
world. agg returns to conv world via per-channel SBUF->SBUF scatter DMAs
streamed per head-group.

The fusion conv (2C->C 3x3, ~70% of all FLOPs) is split into its x-half
(input channels 0..511) and agg-half (512..1023).  The x-half depends
only on x, so its matmuls are emitted interleaved with the attention
head-groups: they fill every tensor-engine gap, keeping the PE HAM
clock-gate warm (the v1 kernel ran the whole attention phase at 1.2GHz
because of ~4us PE idle gaps).  x-half partials bounce through DRAM in
bf16 (contiguous, cheap) and are re-added during the agg-half pass,
which also computes BN stats per chunk so the tail is just
AllReduce -> fused Silu activation -> store.

Spatial layout "w65": each row padded to 65 elements with a LEADING zero
(index d*65 holds 0, data at d*65+1 .. +64, plus 4 trailing zeros; total
4164). A 3x3 tap (dy,dx) then reads a plain strided AP at offset
(r+dy-1)*65 + dx; horizontal SAME-padding is automatic, vertical padding
via per-tap row clamping.
"""

import os
import sys

for _p in ("/opt/trn_rl_repo", os.path.dirname(os.path.abspath(__file__))):
    if _p not in sys.path:
        sys.path.insert(0, _p)

import numpy as np
import ml_dtypes

import concourse.tile as _tile_mod


def _apply_toolchain_patches():
    """This container's walrus accepts at most ONE sync-wait per
    instruction; Tile's exit drain and scheduler attach several. Split the
    exit drain into single-wait drains, and post-process the module to
    hoist extra waits onto same-engine NoOps."""
    import concourse.mybir as mybir

    def _split_drain_and_barrier(self, tick_clock, wait_clock):
        from concourse.tile import ScopedClock
        nc = self.nc
        drain_inst = nc.sync.drain()
        wait_clock.add_sem_waits(
            drain_inst.ins, ScopedClock({None: tick_clock.global_clock}))
        si = drain_inst.ins.sync_info
        if si is not None and len(si.on_wait) > 1:
            waits = list(si.on_wait)
            drain_inst.ins.sync_info = type(si)(
                on_wait=waits[:1], on_update=list(si.on_update))
            for w in waits[1:]:
                d2 = nc.sync.drain()
                si2 = d2.ins.sync_info
                if si2 is None:
                    d2.ins.sync_info = type(si)(on_wait=[w], on_update=[])
                else:
                    d2.ins.sync_info = type(si2)(
                        on_wait=list(si2.on_wait) + [w],
                        on_update=list(si2.on_update))
        nc.all_engine_barrier()
        assert self.sems is not None
        popped = nc._tile_sem_poison_stack.pop()
        assert popped is self._sem_poison
        nc.clear_and_free_semaphores(list(self.sems.allocated().values()))
        nc.all_engine_barrier()

    _tile_mod.TileContext._drain_and_barrier = _split_drain_and_barrier


def _split_multi_waits(nc):
    import concourse.mybir as mybir
    n_split = 0
    for fn in nc.m.functions:
        for bb in fn.blocks:
            insts = list(bb.instructions)
            out = []
            changed = False
            for inst in insts:
                si = inst.sync_info
                if si is not None and len(si.on_wait) > 1:
                    waits = list(si.on_wait)
                    for w in waits[:-1]:
                        nop = mybir.InstNoOp(
                            name=f"{inst.name}-wsplit{n_split}",
                            engine=inst.engine, bass_nofuse=True)
                        nop.sync_info = mybir.SyncInfo(on_wait=[w], on_update=[])
                        out.append(nop)
                        n_split += 1
                    inst.sync_info = type(si)(
                        on_wait=[waits[-1]], on_update=list(si.on_update))
                    changed = True
                out.append(inst)
            if changed:
                bb.instructions = out
    return n_split


_apply_toolchain_patches()

import concourse.bass as bass  # noqa: E402
import concourse.tile as tile  # noqa: E402
from concourse import mybir  # noqa: E402
from concourse.bass_utils import run_bass_kernel_spmd  # noqa: E402

F32 = mybir.dt.float32
BF16 = mybir.dt.bfloat16

B, C, H, W = 8, 512, 64, 64
NH, HD = 8, 64
EPS = 1e-5
W65 = 65
SP65 = 64 * W65 + 4  # 4164
SP = 4096
N_CORES = 8

LAST_RESULTS = None


def _ap(t, offset, dims):
    return bass.AP(tensor=t.tensor, offset=t.offset + offset, ap=[t.ap[0]] + dims)


def _slice_part(t, p0, np_, offset, dims):
    # tile[p0:p0+np_] then rebuild free dims
    sub = t[p0:p0 + np_]
    return bass.AP(tensor=sub.tensor, offset=sub.offset + offset,
                   ap=[sub.ap[0]] + dims)


TAPS = [(1, 0), (1, 1), (1, 2), (0, 0), (0, 1), (0, 2), (2, 0), (2, 1), (2, 2)]


def conv3x3_chunk_gen(nc, psum, w_tiles, src_tiles, r0, n_ktiles, co=128):
    """3x3 conv, one 8-row chunk, accumulated into psum (co, 8*64).
    w_tiles[(tap_idx, kt)] = lhsT (K, co); src_tiles[kt] = w65 tile.
    dy=1 taps first (full coverage -> start=True clears the bank).
    Yields after every 4 matmuls so callers can interleave other PE work."""
    mms = []
    for dy, dx in TAPS:
        t_i = dy * 3 + dx  # host weight tap order is (ky, kx) row-major
        a = max(r0, 1 - dy)
        b = min(r0 + 8, 65 - dy)
        nr = b - a
        if nr <= 0:
            continue
        for kt in range(n_ktiles):
            mms.append((t_i, kt, a, nr, dy, dx))
    for j, (t_i, kt, a, nr, dy, dx) in enumerate(mms):
        src = src_tiles[kt]
        in_ap = _ap(src, (a + dy - 1) * W65 + dx, [[W65, nr], [1, 64]])
        nc.tensor.matmul(
            psum[0:co, (a - r0) * 64:(a - r0 + nr) * 64], w_tiles[(t_i, kt)],
            in_ap, start=(j == 0), stop=(j == len(mms) - 1),
            skip_group_check=True)
        if (j + 1) % 4 == 0:
            yield


def conv3x3_chunk(nc, psum, w_tiles, src_tiles, r0, n_ktiles, co=128):
    for _ in conv3x3_chunk_gen(nc, psum, w_tiles, src_tiles, r0, n_ktiles, co):
        pass


def build_program():
    nc = bass.Bass(trn_type="TRN2", num_devices=N_CORES)

    d_x65 = nc.dram_tensor("x65", [C, SP65], BF16, kind="ExternalInput")
    d_xT = nc.dram_tensor("xT", [C, SP], BF16, kind="ExternalInput")
    d_wqkT = nc.dram_tensor("wqkT", [C, 1024], BF16, kind="ExternalInput")
    d_wvT = nc.dram_tensor("wvT", [C, 512], BF16, kind="ExternalInput")
    d_bqk = nc.dram_tensor("bqk", [1, 1024], BF16, kind="ExternalInput")
    d_bv = nc.dram_tensor("bv", [1, 512], BF16, kind="ExternalInput")
    d_bw1 = nc.dram_tensor("bw1", [4, 128, 576], BF16, kind="ExternalInput")
    d_bb1 = nc.dram_tensor("bb1", [64, 1], F32, kind="ExternalInput")
    d_bw2 = nc.dram_tensor("bw2", [64, 4], BF16, kind="ExternalInput")
    d_bb2 = nc.dram_tensor("bb2", [4, 1], F32, kind="ExternalInput")
    d_ew1 = nc.dram_tensor("ew1", [4, 576], BF16, kind="ExternalInput")
    d_eb1 = nc.dram_tensor("eb1", [64, 1], F32, kind="ExternalInput")
    d_ew2 = nc.dram_tensor("ew2", [64, 8], BF16, kind="ExternalInput")
    d_eb2 = nc.dram_tensor("eb2", [8, 1], F32, kind="ExternalInput")
    d_gng = nc.dram_tensor("gng", [64, 1], F32, kind="ExternalInput")
    d_gnb = nc.dram_tensor("gnb", [64, 1], F32, kind="ExternalInput")
    d_gmat = nc.dram_tensor("gmat", [64, 8], F32, kind="ExternalInput")
    d_sel = nc.dram_tensor("sel", [8, 512], BF16, kind="ExternalInput")
    d_fw2 = nc.dram_tensor("fw2", [2, 4, 128, 4608], BF16, kind="ExternalInput")
    d_fb = nc.dram_tensor("fb", [C, 1], F32, kind="ExternalInput")
    d_bng = nc.dram_tensor("bng", [C, 1], F32, kind="ExternalInput")
    d_bnb = nc.dram_tensor("bnb", [C, 1], F32, kind="ExternalInput")
    d_y = nc.dram_tensor("y", [C, SP], F32, kind="ExternalOutput")

    AF = mybir.ActivationFunctionType

    with tile.TileContext(nc) as tc:
        with tc.tile_pool(name="glob", bufs=1) as glob, \
             tc.tile_pool(name="psA", bufs=3, space="PSUM") as psA, \
             tc.tile_pool(name="psB", bufs=3, space="PSUM") as psB, \
             tc.tile_pool(name="psF", bufs=2, space="PSUM") as psF, \
             tc.tile_pool(name="dram", bufs=1, space="DRAM") as dram:

            x65 = [glob.tile([128, SP65], BF16, name=f"x65_{k}") for k in range(4)]
            ones1 = glob.tile([1, 128], BF16)
            for k in range(4):
                # split across both HWDGE trigger queues to halve load time
                eng = nc.sync if k < 2 else nc.scalar
                eng.dma_start(out=x65[k], in_=d_x65[k * 128:(k + 1) * 128, :])
            nc.vector.memset(ones1, 1.0)
            agg65 = [glob.tile([128, SP65], BF16, name=f"agg65_{k}")
                     for k in range(4)]
            for k in range(4):
                nc.vector.memset(agg65[k], 0.0)
            xT = [glob.tile([128, SP], BF16, name=f"xT_{k}") for k in range(4)]
            sel8 = glob.tile([8, 512], BF16)
            nc.sync.dma_start(out=sel8, in_=d_sel[:])
            edge_fl = glob.tile([8, SP], BF16)
            fb = glob.tile([128, 4], F32)
            nc.sync.dma_start(
                out=fb, in_=d_fb[:].rearrange("(a b) c -> b (a c)", a=4))
            bng = glob.tile([128, 4], F32)
            nc.sync.dma_start(
                out=bng, in_=d_bng[:].rearrange("(a b) c -> b (a c)", a=4))
            bnb = glob.tile([128, 4], F32)
            nc.sync.dma_start(
                out=bnb, in_=d_bnb[:].rearrange("(a b) c -> b (a c)", a=4))
            stats_l = glob.tile([128, 8], F32)
            d_ytx = dram.tile([C, SP], BF16)  # fusion x-half partial bounce
            d_aggB = [dram.tile([64, 8192], BF16, name=f"aggB_{g}")
                      for g in range(4)]  # agg bounce, (d, c_loc*64+w)

            # fusion conv weight streaming pool (lives across ph2 + ph3)
            with tc.tile_pool(name="fwp", bufs=1) as fwp:

                def load_fw(half, ct):
                    """One packed DMA for all 36 lhsT tiles of a fusion-conv
                    (half, ct) group; returns the (tap, kt)->lhsT AP dict."""
                    fw_all = fwp.tile([128, 4608], BF16, tag="fwall", bufs=2,
                                      name=f"fwall_{half}_{ct}")
                    nc.sync.dma_start(out=fw_all, in_=d_fw2[half, ct])
                    return {(t_i, kt): fw_all[:, (t_i * 4 + kt) * 128:
                                              (t_i * 4 + kt + 1) * 128]
                            for t_i in range(9) for kt in range(4)}

                def emit_xhalf_ct(ct):
                    """Fusion-conv x-half (input channels 0..511) for output
                    channel tile ct. PE filler work: depends only on x65 +
                    weights. Result (incl. fusion bias) -> d_ytx in bf16.
                    Generator: yields every 4 matmuls for PE interleaving."""
                    fw = load_fw(0, ct)
                    for ch in range(8):
                        pf = psF.tile([128, 512], F32, tag="fx",
                                      name=f"pfx_{ct}_{ch}")
                        yield from conv3x3_chunk_gen(nc, pf, fw, x65,
                                                     ch * 8, 4)
                        st = fwp.tile([128, 512], BF16, tag="ytxs", bufs=4,
                                      name=f"ytxs_{ct}_{ch}")
                        nc.vector.tensor_scalar_add(out=st, in0=pf,
                                                    scalar1=fb[:, ct:ct + 1])
                        nc.sync.dma_start(
                            out=d_ytx[ct * 128:(ct + 1) * 128,
                                      ch * 512:(ch + 1) * 512],
                            in_=st)

                class _Fillers:
                    """Doles out x-half fusion matmul quanta (4 MMs each) as
                    PE filler, consuming ct groups 0..3 in order."""
                    def __init__(self):
                        self.next_ct = 0
                        self.cur = None

                    def step(self, n=1):
                        for _ in range(n):
                            while True:
                                if self.cur is None:
                                    if self.next_ct >= 4:
                                        return
                                    self.cur = emit_xhalf_ct(self.next_ct)
                                    self.next_ct += 1
                                try:
                                    next(self.cur)
                                    break
                                except StopIteration:
                                    self.cur = None

                    def drain_all(self):
                        while self.cur is not None or self.next_ct < 4:
                            if self.cur is None:
                                self.cur = emit_xhalf_ct(self.next_ct)
                                self.next_ct += 1
                            for _ in self.cur:
                                pass
                            self.cur = None

                fillers = _Fillers()

                # ---------- Phase 1: box_net + edge_net ----------
                if True:
                    with tc.tile_pool(name="p1", bufs=1) as p1:
                        bw1_all = [p1.tile([128, 576], BF16, name=f"bw1a_{kt}")
                                   for kt in range(4)]
                        for kt in range(4):
                            nc.sync.dma_start(out=bw1_all[kt], in_=d_bw1[kt])
                        bw1 = {(t_i, kt):
                               bw1_all[kt][:, t_i * 64:(t_i + 1) * 64]
                               for t_i in range(9) for kt in range(4)}
                        bb1 = p1.tile([64, 1], F32)
                        nc.sync.dma_start(out=bb1, in_=d_bb1[:])
                        bw2 = p1.tile([64, 4], BF16)
                        nc.sync.dma_start(out=bw2, in_=d_bw2[:])
                        bb2 = p1.tile([4, 1], F32)
                        nc.sync.dma_start(out=bb2, in_=d_bb2[:])
                        ew1_all = p1.tile([4, 576], BF16, name="ew1a")
                        nc.sync.dma_start(out=ew1_all, in_=d_ew1[:])
                        ew1 = {(t_i, 0): ew1_all[:, t_i * 64:(t_i + 1) * 64]
                               for t_i in range(9)}
                        eb1 = p1.tile([64, 1], F32)
                        nc.sync.dma_start(out=eb1, in_=d_eb1[:])
                        ew2 = p1.tile([64, 8], BF16)
                        nc.sync.dma_start(out=ew2, in_=d_ew2[:])
                        eb2 = p1.tile([8, 1], F32)
                        nc.sync.dma_start(out=eb2, in_=d_eb2[:])
                        gng = p1.tile([64, 1], F32)
                        nc.sync.dma_start(out=gng, in_=d_gng[:])
                        gnb = p1.tile([64, 1], F32)
                        nc.sync.dma_start(out=gnb, in_=d_gnb[:])
                        gmat = p1.tile([64, 8], F32)
                        nc.sync.dma_start(out=gmat, in_=d_gmat[:])
                        # xT loads deferred here: box weights reach SBUF
                        # first so the box conv starts ~15us earlier
                        for k in range(4):
                            nc.sync.dma_start(
                                out=xT[k], in_=d_xT[k * 128:(k + 1) * 128, :])

                        box1 = p1.tile([64, SP65], BF16)
                        nc.vector.memset(box1, 0.0)
                        for ch in range(8):
                            pb = psA.tile([128, 512], F32, tag="conv",
                                          name=f"pb_{ch}")
                            conv3x3_chunk(nc, pb, bw1, x65, ch * 8, 4, co=64)
                            nc.scalar.activation(
                                out=_slice_part(box1, 0, 64, ch * 8 * W65 + 1,
                                                [[W65, 8], [1, 64]]),
                                in_=pb[0:64, :], func=AF.Gelu, bias=bb1,
                                scale=1.0)

                        boxes = p1.tile([4, SP65], BF16)
                        nc.vector.memset(boxes, 0.0)
                        for ch in range(8):
                            pb2 = psA.tile([128, 512], F32, tag="conv",
                                           name=f"pb2_{ch}")
                            nc.tensor.matmul(
                                pb2[0:4, :], bw2,
                                _ap(box1, ch * 8 * W65 + 1, [[W65, 8], [1, 64]]),
                                start=True, stop=True)
                            nc.scalar.activation(
                                out=_slice_part(boxes, 0, 4, ch * 8 * W65 + 1,
                                                [[W65, 8], [1, 64]]),
                                in_=pb2[0:4, :], func=AF.Sigmoid, bias=bb2,
                                scale=1.0)
                            fillers.step(2)

                        e1 = p1.tile([64, SP], F32)
                        for ch in range(8):
                            pe = psA.tile([128, 512], F32, tag="conv",
                                          name=f"pe_{ch}")
                            conv3x3_chunk(nc, pe, ew1, [boxes], ch * 8, 1, co=64)
                            nc.scalar.activation(
                                out=e1[:, ch * 512:(ch + 1) * 512],
                                in_=pe[0:64, :],
                                func=AF.Identity, bias=eb1, scale=1.0)
                            fillers.step(2)

                        stats = p1.tile([64, 8, 6], F32)
                        for j in range(8):
                            nc.vector.bn_stats(out=stats[:, j, :],
                                               in_=e1[:, j * 512:(j + 1) * 512])
                        mv = p1.tile([64, 2], F32)
                        nc.vector.bn_aggr(out=mv, in_=stats)
                        ex2 = p1.tile([64, 2], F32)
                        nc.vector.tensor_copy(out=ex2[:, 0:1], in_=mv[:, 0:1])
                        nc.vector.tensor_mul(out=ex2[:, 1:2], in0=mv[:, 0:1],
                                             in1=mv[:, 0:1])
                        nc.vector.tensor_add(out=ex2[:, 1:2], in0=ex2[:, 1:2],
                                             in1=mv[:, 1:2])
                        gs_ps = psB.tile([8, 2], F32, tag="att", name="gs_ps")
                        nc.tensor.matmul(gs_ps, gmat, ex2, start=True, stop=True)
                        gs = p1.tile([8, 2], F32)
                        nc.scalar.activation(out=gs, in_=gs_ps, func=AF.Copy,
                                             bias=0.0, scale=1.0 / 8.0)
                        gvar = p1.tile([8, 1], F32)
                        eps8 = p1.tile([8, 1], F32)
                        nc.vector.memset(eps8, float(EPS))
                        nc.vector.tensor_mul(out=gvar, in0=gs[:, 0:1],
                                             in1=gs[:, 0:1])
                        nc.vector.tensor_sub(out=gvar, in0=gs[:, 1:2], in1=gvar)
                        nc.scalar.activation(out=gvar, in_=gvar, func=AF.Sqrt,
                                             bias=eps8, scale=1.0)
                        nc.vector.reciprocal(out=gvar, in_=gvar)
                        gmr = p1.tile([8, 2], F32)
                        nc.vector.tensor_copy(out=gmr[:, 0:1], in_=gs[:, 0:1])
                        nc.vector.tensor_copy(out=gmr[:, 1:2], in_=gvar)
                        cmr = p1.tile([64, 2], F32)
                        src_bc = bass.AP(tensor=gmr.tensor, offset=gmr.offset,
                                         ap=[[gmr.ap[0][0], 8], [0, 8], [1, 2]])
                        nc.sync.dma_start(out=cmr, in_=src_bc)
                        gsc = p1.tile([64, 1], F32)
                        nc.vector.tensor_mul(out=gsc, in0=cmr[:, 1:2], in1=gng)
                        gsh = p1.tile([64, 1], F32)
                        nc.vector.tensor_mul(out=gsh, in0=cmr[:, 0:1], in1=gsc)
                        nc.vector.tensor_sub(out=gsh, in0=gnb, in1=gsh)
                        fillers.step(6)
                        e1g = p1.tile([64, SP], BF16)
                        nc.scalar.activation(out=e1g, in_=e1, func=AF.Gelu,
                                             bias=gsh, scale=gsc)
                        for ch in range(8):
                            pe2 = psA.tile([128, 512], F32, tag="conv",
                                           name=f"pe2_{ch}")
                            nc.tensor.matmul(pe2[0:8, :], ew2,
                                             e1g[:, ch * 512:(ch + 1) * 512],
                                             start=True, stop=True)
                            # store exp(edge): scores use
                            # exp(S+e) = exp(S)*exp(e), saving a K=8
                            # (near-zero PE duty) matmul per score block
                            nc.scalar.activation(
                                out=edge_fl[:, ch * 512:(ch + 1) * 512],
                                in_=pe2[0:8, :], func=AF.Exp, bias=eb2,
                                scale=1.0)
                            fillers.step(2)

                # ---------- Phase 2: qkv conv + attention ----------
                with tc.tile_pool(name="p2", bufs=1) as p2:
                    wqkT = [p2.tile([128, 1024], BF16, name=f"wqkT_{kt}")
                            for kt in range(4)]
                    wvT = [p2.tile([128, 512], BF16, name=f"wvT_{kt}")
                           for kt in range(4)]
                    for kt in range(4):
                        nc.sync.dma_start(out=wqkT[kt],
                                          in_=d_wqkT[kt * 128:(kt + 1) * 128, :])
                        nc.sync.dma_start(out=wvT[kt],
                                          in_=d_wvT[kt * 128:(kt + 1) * 128, :])
                    bqk = p2.tile([1, 1024], BF16)
                    nc.sync.dma_start(out=bqk, in_=d_bqk[:])
                    bv = p2.tile([1, 512], BF16)
                    nc.sync.dma_start(out=bv, in_=d_bv[:])
                    # partition-broadcast copies: bias rides the psum->SBUF
                    # vector adds instead of K=1 matmuls (those stream N
                    # cycles at ~zero PE-array duty and cool the HAM gate)
                    bqk64 = p2.tile([64, 1024], BF16)
                    nc.sync.dma_start(
                        out=bqk64,
                        in_=bass.AP(tensor=bqk.tensor, offset=bqk.offset,
                                    ap=[bqk.ap[0], [0, 64], [1, 1024]]))
                    bv64 = p2.tile([64, 512], BF16)
                    nc.sync.dma_start(
                        out=bv64,
                        in_=bass.AP(tensor=bv.tensor, offset=bv.offset,
                                    ap=[bv.ap[0], [0, 64], [1, 512]]))

                    for g in range(4):
                        # qka and va share one slot (tag "qv"): qka dies
                        # after the score matmuls, va is built right after.
                        qka = p2.tile([64, 64, 256], BF16, tag="qv",
                                      name=f"qka_{g}")
                        # qk conv: x^T stationary, column pairs
                        for sp in range(32):
                            pq = psA.tile([128, 256], F32, tag="conv",
                                          name=f"pq_{g}_{sp}")
                            for kt in range(4):
                                nc.tensor.matmul(
                                    pq, xT[kt][:, sp * 128:(sp + 1) * 128],
                                    wqkT[kt][:, g * 256:(g + 1) * 256],
                                    start=(kt == 0), stop=(kt == 3))
                            for j in range(2):
                                nc.vector.tensor_add(
                                    out=_ap(qka, (2 * sp + j) * 256, [[1, 256]]),
                                    in0=pq[64 * j:64 * (j + 1), :],
                                    in1=bqk64[:, g * 256:(g + 1) * 256])
                        # logits + exp, per head
                        Pn = [None, None]
                        eedge = [None, None]
                        for hh in range(2):
                            h = 2 * g + hh
                            qoff, koff = hh * 128, hh * 128 + 64
                            # broadcast exp(edge) row h across partitions
                            # (shares the "blk" slot pair; freed by the
                            # softmax multiply before AV needs the slot)
                            eedge[hh] = p2.tile([64, SP], BF16, tag="blk",
                                                name=f"eedge_{g}_{hh}")
                            sub = edge_fl[h:h + 1]
                            nc.sync.dma_start(
                                out=eedge[hh],
                                in_=bass.AP(tensor=sub.tensor,
                                            offset=sub.offset,
                                            ap=[sub.ap[0], [0, 64], [1, SP]]))
                            Pn[hh] = p2.tile([64, SP], BF16, tag="Pn",
                                             bufs=2, name=f"Pn_{g}_{hh}")
                            for ib in range(8):
                                sp_ = psB.tile([64, 512], F32, tag="att",
                                               name=f"sp_{g}_{hh}_{ib}")
                                for ii in range(8):
                                    i = ib * 8 + ii
                                    nc.tensor.matmul(
                                        sp_[:, ii * 64:(ii + 1) * 64],
                                        _ap(qka, koff + i, [[256, 64]]),
                                        _ap(qka, qoff + i, [[256, 64]]),
                                        start=(ii == 0), stop=(ii == 7),
                                        skip_group_check=True)
                                nc.scalar.activation(
                                    out=_ap(Pn[hh], ib * 512, [[1, 512]]),
                                    in_=sp_, func=AF.Exp)
                        # v conv: x stationary, one row per matmul (M=64;
                        # walrus allows only one free dim on the stationary
                        # operand), with x-half fusion matmuls interleaved
                        # as PE filler so the HAM clock-gate stays warm
                        va = p2.tile([64, 64, 128], BF16, tag="qv",
                                     name=f"va_{g}")
                        for sp in range(32):
                            for j in range(2):
                                d0 = 2 * sp + j
                                pv = psA.tile([64, 128], F32, tag="conv",
                                              name=f"pv_{g}_{d0}")
                                for kt in range(4):
                                    lhs = _ap(x65[kt], d0 * W65 + 1, [[1, 64]])
                                    nc.tensor.matmul(
                                        pv, lhs,
                                        wvT[kt][:, g * 128:(g + 1) * 128],
                                        start=(kt == 0), stop=(kt == 3))
                                nc.vector.tensor_add(
                                    out=_ap(va, d0 * 128, [[1, 128]]), in0=pv,
                                    in1=bv64[:, g * 128:(g + 1) * 128])
                                # interleave x-half fusion matmuls as PE
                                # filler (HAM warmth); hold some back on the
                                # last group to cover the phase-3 boundary
                                if g < 2 or j == 0:
                                    fillers.step(1)
                        # softmax normalize + AV per head
                        for hh in range(2):
                            h = 2 * g + hh
                            Px = Pn[hh]
                            nc.vector.tensor_mul(out=Px, in0=Px,
                                                 in1=eedge[hh])
                            D = p2.tile([64, 64], F32, tag=f"D{hh}",
                                        name=f"D_{g}_{hh}")
                            Pv = bass.AP(tensor=Px.tensor, offset=Px.offset,
                                         ap=[Px.ap[0], [1, 64], [64, 64]])
                            nc.vector.reduce_sum(out=D, in_=Pv,
                                                 axis=mybir.AxisListType.X)
                            nc.vector.reciprocal(out=D, in_=D)
                            Rb = bass.AP(tensor=D.tensor, offset=D.offset,
                                         ap=[D.ap[0], [0, 64], [1, 64]])
                            nc.vector.tensor_mul(out=Px, in0=Px, in1=Rb)
                            blk = p2.tile([64, SP], BF16, tag="blk",
                                          name=f"blk_{g}_{hh}")
                            for ib in range(8):
                                ap2 = psB.tile([64, 512], F32, tag="att",
                                               name=f"ap2_{g}_{hh}_{ib}")
                                for ii in range(8):
                                    i = ib * 8 + ii
                                    nc.tensor.matmul(
                                        ap2[:, ii * 64:(ii + 1) * 64],
                                        _ap(va, hh * 64 + i, [[128, 64]]),
                                        Px[:, i * 64:(i + 1) * 64],
                                        start=(ii == 0), stop=(ii == 7),
                                        skip_group_check=True)
                                nc.vector.tensor_copy(
                                    out=blk[:, ib * 512:(ib + 1) * 512],
                                    in_=ap2)
                            # contiguous write: (d, c_loc*64+w) block layout
                            nc.sync.dma_start(
                                out=_ap(d_aggB[g], hh * 4096, [[1, 4096]]),
                                in_=blk)
                            # agg -> conv world: strided read-back per head
                            # (starts during the other head's AV)
                            nc.sync.dma_start(
                                out=_slice_part(agg65[g], hh * 64, 64, 1,
                                                [[W65, 64], [1, 64]]),
                                in_=bass.AP(
                                    tensor=d_aggB[g].tensor,
                                    offset=d_aggB[g].offset + hh * 4096,
                                    ap=[[64, 64], [8192, 64], [1, 64]]))
                        # after the last group, drain all remaining x-half
                        # filler work to cover the agg read-back latency
                        if g == 3:
                            fillers.drain_all()

                # ================= Phase 3: agg-half + BN =================
                with tc.tile_pool(name="p3", bufs=1) as p3:
                    ypre = [p3.tile([128, SP], BF16, name=f"ypre_{k}")
                            for k in range(4)]
                    eps128 = p3.tile([128, 1], F32)
                    nc.vector.memset(eps128, float(EPS))
                    for ct in range(4):
                        fw = load_fw(1, ct)
                        st6 = p3.tile([128, 8, 6], F32, tag="st6",
                                      name=f"st6_{ct}")
                        for ch in range(8):
                            ytr = fwp.tile([128, 512], BF16, tag="ytxr",
                                           bufs=3, name=f"ytxr_{ct}_{ch}")
                            nc.sync.dma_start(
                                out=ytr,
                                in_=d_ytx[ct * 128:(ct + 1) * 128,
                                          ch * 512:(ch + 1) * 512])
                            pf = (psA if ch % 2 == 0 else psF).tile(
                                [128, 512], F32,
                                tag=("conv" if ch % 2 == 0 else "fx"),
                                name=f"pfa_{ct}_{ch}")
                            conv3x3_chunk(nc, pf, fw, agg65, ch * 8, 4)
                            yslice = ypre[ct][:, ch * 512:(ch + 1) * 512]
                            nc.vector.tensor_add(out=yslice, in0=pf, in1=ytr)
                            nc.vector.bn_stats(out=st6[:, ch, :], in_=yslice)
                        mv4 = p3.tile([128, 2], F32, tag="mv4", name=f"mv4_{ct}")
                        nc.vector.bn_aggr(out=mv4, in_=st6)
                        nc.scalar.activation(out=stats_l[:, 2 * ct:2 * ct + 1],
                                             in_=mv4[:, 0:1], func=AF.Copy,
                                             bias=0.0, scale=float(SP))
                        sq = p3.tile([128, 1], F32, tag="sq", name=f"sq_{ct}")
                        nc.vector.tensor_mul(out=sq, in0=mv4[:, 0:1],
                                             in1=mv4[:, 0:1])
                        nc.vector.tensor_add(out=sq, in0=sq, in1=mv4[:, 1:2])
                        nc.scalar.activation(
                            out=stats_l[:, 2 * ct + 1:2 * ct + 2],
                            in_=sq, func=AF.Copy, bias=0.0, scale=float(SP))

                    NTOT = float(B * SP)

                    def bn_apply(cts, rstats, ncols):
                        """AllReduced (sum, sumsq) cols -> sc/sh -> fused
                        Silu+store for the given ct list."""
                        mean = p3.tile([128, ncols], F32, tag=f"mn{ncols}")
                        nc.scalar.activation(out=mean,
                                             in_=_ap(rstats, 0, [[2, ncols]]),
                                             func=AF.Copy, bias=0.0,
                                             scale=1.0 / NTOT)
                        e2 = p3.tile([128, ncols], F32, tag=f"e2{ncols}")
                        nc.scalar.activation(out=e2,
                                             in_=_ap(rstats, 1, [[2, ncols]]),
                                             func=AF.Copy, bias=0.0,
                                             scale=1.0 / NTOT)
                        var = p3.tile([128, ncols], F32, tag=f"vr{ncols}")
                        nc.vector.tensor_mul(out=var, in0=mean, in1=mean)
                        nc.vector.tensor_sub(out=var, in0=e2, in1=var)
                        nc.scalar.activation(out=var, in_=var, func=AF.Sqrt,
                                             bias=eps128, scale=1.0)
                        nc.vector.reciprocal(out=var, in_=var)
                        sc = p3.tile([128, ncols], F32, tag=f"sc{ncols}")
                        nc.vector.tensor_mul(out=sc, in0=var,
                                             in1=bng[:, cts[0]:cts[0] + ncols])
                        sh = p3.tile([128, ncols], F32, tag=f"sh{ncols}")
                        nc.vector.tensor_mul(out=sh, in0=mean, in1=sc)
                        nc.vector.tensor_sub(out=sh,
                                             in0=bnb[:, cts[0]:cts[0] + ncols],
                                             in1=sh)
                        for ci, ct in enumerate(cts):
                            for half in range(2):
                                sg = p3.tile([128, SP // 2], F32, tag="sg",
                                             bufs=3, name=f"sg_{ct}_{half}")
                                nc.scalar.activation(
                                    out=sg,
                                    in_=ypre[ct][:, half * 2048:
                                                 (half + 1) * 2048],
                                    func=AF.Silu, bias=sh[:, ci:ci + 1],
                                    scale=sc[:, ci:ci + 1])
                                nc.sync.dma_start(
                                    out=d_y[ct * 128:(ct + 1) * 128,
                                            half * 2048:(half + 1) * 2048],
                                    in_=sg)

                    # split AllReduce: cts 0-2 fire after ct2's stats and
                    # hide (CC + BN math + Silu + store) under ct3's conv;
                    # only ct3's small CC is exposed at the tail.
                    cc_in_a = dram.tile([128, 6], F32)
                    cc_out_a = dram.tile([128, 6], F32)
                    nc.sync.dma_start(out=cc_in_a, in_=stats_l[:, 0:6])
                    nc.gpsimd.collective_compute(
                        "AllReduce", mybir.AluOpType.add,
                        replica_groups=[list(range(N_CORES))],
                        ins=[cc_in_a.opt()], outs=[cc_out_a.opt()])
                    rstats_a = p3.tile([128, 6], F32)
                    nc.sync.dma_start(out=rstats_a, in_=cc_out_a)
                    bn_apply([0, 1, 2], rstats_a, 3)

                    cc_in_b = dram.tile([128, 2], F32)
                    cc_out_b = dram.tile([128, 2], F32)
                    nc.sync.dma_start(out=cc_in_b, in_=stats_l[:, 6:8])
                    nc.gpsimd.collective_compute(
                        "AllReduce", mybir.AluOpType.add,
                        replica_groups=[list(range(N_CORES))],
                        ins=[cc_in_b.opt()], outs=[cc_out_b.opt()])
                    rstats_b = p3.tile([128, 2], F32)
                    nc.sync.dma_start(out=rstats_b, in_=cc_out_b)
                    bn_apply([3], rstats_b, 1)

    _split_multi_waits(nc)
    return nc


_PROGRAM = None


def _get_program():
    global _PROGRAM
    if _PROGRAM is None:
        _PROGRAM = build_program()
    return _PROGRAM


def _bf16(a):
    return np.ascontiguousarray(np.asarray(a, np.float32).astype(ml_dtypes.bfloat16))


def _f32(a):
    return np.ascontiguousarray(np.asarray(a, np.float32))


def kernel(x, box_w1, box_b1, box_w2, box_b2, edge_w1, edge_b1, gn_g, gn_b,
           edge_w2, edge_b2, qkv_w, qkv_b, fus_w, fus_b, bn_g, bn_b,
           trace=False):
    global LAST_RESULTS
    x = np.asarray(x, np.float32)
    scale = float(HD) ** -0.5

    qkv_w2 = np.asarray(qkv_w, np.float32).reshape(3 * C, C)
    qkv_b2 = np.asarray(qkv_b, np.float32).copy()
    wq = qkv_w2[0:C] * scale
    bq = qkv_b2[0:C] * scale
    wk, bk = qkv_w2[C:2 * C], qkv_b2[C:2 * C]
    wv, bv_ = qkv_w2[2 * C:], qkv_b2[2 * C:]
    wqk = np.empty((1024, C), np.float32)
    bqk = np.empty(1024, np.float32)
    for h in range(NH):
        wqk[h * 128:h * 128 + 64] = wq[h * 64:(h + 1) * 64]
        wqk[h * 128 + 64:(h + 1) * 128] = wk[h * 64:(h + 1) * 64]
        bqk[h * 128:h * 128 + 64] = bq[h * 64:(h + 1) * 64]
        bqk[h * 128 + 64:(h + 1) * 128] = bk[h * 64:(h + 1) * 64]

    bw1T = np.asarray(box_w1, np.float32).transpose(2, 3, 1, 0).reshape(9, C, 64)
    # packed box weights: bw1p[kt][p, t*64+co] = bw1T[t, kt*128+p, co]
    bw1p = np.ascontiguousarray(
        bw1T.reshape(9, 4, 128, 64).transpose(1, 2, 0, 3).reshape(4, 128, 576))
    ew1T = np.asarray(edge_w1, np.float32).transpose(2, 3, 1, 0).reshape(9, 4, 64)
    ew1p = np.ascontiguousarray(
        ew1T.transpose(1, 0, 2).reshape(4, 576))
    fwT = np.asarray(fus_w, np.float32).transpose(2, 3, 1, 0).reshape(9, 1024, C)
    fwT_t = fwT.reshape(9, 8, 128, 4, 128).transpose(0, 1, 3, 2, 4)
    # packed fusion weights: fw2[half, ct, p, (t*4+ktl)*128+co]
    fw2 = np.ascontiguousarray(
        fwT_t.reshape(9, 2, 4, 4, 128, 128).transpose(1, 3, 4, 0, 2, 5)
        .reshape(2, 4, 128, 4608))

    gmat = np.zeros((64, 8), np.float32)
    for g in range(8):
        gmat[g * 8:(g + 1) * 8, g] = 1.0

    shared = {
        "wqkT": _bf16(wqk.T), "wvT": _bf16(wv.T),
        "bqk": _bf16(bqk[None, :]), "bv": _bf16(bv_[None, :]),
        "bw1": _bf16(bw1p), "bb1": _f32(np.asarray(box_b1).reshape(64, 1)),
        "bw2": _bf16(np.asarray(box_w2, np.float32).reshape(4, 64).T),
        "bb2": _f32(np.asarray(box_b2).reshape(4, 1)),
        "ew1": _bf16(ew1p), "eb1": _f32(np.asarray(edge_b1).reshape(64, 1)),
        "ew2": _bf16(np.asarray(edge_w2, np.float32).reshape(8, 64).T),
        "eb2": _f32(np.asarray(edge_b2).reshape(8, 1)),
        "gng": _f32(np.asarray(gn_g).reshape(64, 1)),
        "gnb": _f32(np.asarray(gn_b).reshape(64, 1)),
        "gmat": gmat,
        "sel": _bf16(np.kron(np.eye(8, dtype=np.float32),
                             np.ones((1, 64), np.float32))),
        "fw2": _bf16(fw2),
        "fb": _f32(np.asarray(fus_b).reshape(C, 1)),
        "bng": _f32(np.asarray(bn_g).reshape(C, 1)),
        "bnb": _f32(np.asarray(bn_b).reshape(C, 1)),
    }

    in_maps = []
    for b in range(B):
        xb = x[b]
        x65h = np.zeros((C, 64, W65), np.float32)
        x65h[:, :, 1:] = xb
        x65h = np.concatenate(
            [x65h.reshape(C, 4160), np.zeros((C, 4), np.float32)], axis=1)
        xTh = np.ascontiguousarray(xb.transpose(0, 2, 1)).reshape(C, SP)
        m = dict(shared)
        m["x65"] = _bf16(x65h)
        m["xT"] = _bf16(xTh)
        in_maps.append(m)

    nc = _get_program()
    res = run_bass_kernel_spmd(nc, in_maps, core_ids=list(range(N_CORES)),
                               trace=trace)
    LAST_RESULTS = res
    out = np.empty((B, C, H, W), np.float32)
    for b in range(B):
        out[b] = res.results[b]["y"].reshape(C, H, W)
    return out


# revision 49
# speedup vs baseline: 1.0069x; 1.0069x over previous
"""AutoBoxGraphAttention Trainium2 kernel (v2 - PE-warm restructure).

Data-parallel over batch: core b handles image b (B=8, one per NeuronCore).
The only cross-core communication is a 4KB AllReduce of BatchNorm partials.

Layout worlds:
  conv world      : channel on partitions, spatial flat on free dim
  attention world : spatial row (d) / col (W') on partitions, per (head, i)
Transitions are free for q,k,v: the qkv 1x1 conv is computed with the
ACTIVATION as the stationary matmul operand (x^T column-pair chunks for
q,k; x row-pair chunks for v), so outputs land directly in attention
world. agg returns to conv world via per-channel SBUF->SBUF scatter DMAs
streamed per head-group.

The fusion conv (2C->C 3x3, ~70% of all FLOPs) is split into its x-half
(input channels 0..511) and agg-half (512..1023).  The x-half depends
only on x, so its matmuls are emitted interleaved with the attention
head-groups: they fill every tensor-engine gap, keeping the PE HAM
clock-gate warm (the v1 kernel ran the whole attention phase at 1.2GHz
because of ~4us PE idle gaps).  x-half partials bounce through DRAM in
bf16 (contiguous, cheap) and are re-added during the agg-half pass,
which also computes BN stats per chunk so the tail is just
AllReduce -> fused Silu activation -> store.

Spatial layout "w65": each row padded to 65 elements with a LEADING zero
(index d*65 holds 0, data at d*65+1 .. +64, plus 4 trailing zeros; total
4164). A 3x3 tap (dy,dx) then reads a plain strided AP at offset
(r+dy-1)*65 + dx; horizontal SAME-padding is automatic, vertical padding
via per-tap row clamping.
"""

import os
import sys

for _p in ("/opt/trn_rl_repo", os.path.dirname(os.path.abspath(__file__))):
    if _p not in sys.path:
        sys.path.insert(0, _p)

import numpy as np
import ml_dtypes

import concourse.tile as _tile_mod


def _apply_toolchain_patches():
    """This container's walrus accepts at most ONE sync-wait per
    instruction; Tile's exit drain and scheduler attach several. Split the
    exit drain into single-wait drains, and post-process the module to
    hoist extra waits onto same-engine NoOps."""
    import concourse.mybir as mybir

    def _split_drain_and_barrier(self, tick_clock, wait_clock):
        from concourse.tile import ScopedClock
        nc = self.nc
        drain_inst = nc.sync.drain()
        wait_clock.add_sem_waits(
            drain_inst.ins, ScopedClock({None: tick_clock.global_clock}))
        si = drain_inst.ins.sync_info
        if si is not None and len(si.on_wait) > 1:
            waits = list(si.on_wait)
            drain_inst.ins.sync_info = type(si)(
                on_wait=waits[:1], on_update=list(si.on_update))
            for w in waits[1:]:
                d2 = nc.sync.drain()
                si2 = d2.ins.sync_info
                if si2 is None:
                    d2.ins.sync_info = type(si)(on_wait=[w], on_update=[])
                else:
                    d2.ins.sync_info = type(si2)(
                        on_wait=list(si2.on_wait) + [w],
                        on_update=list(si2.on_update))
        nc.all_engine_barrier()
        assert self.sems is not None
        popped = nc._tile_sem_poison_stack.pop()
        assert popped is self._sem_poison
        nc.clear_and_free_semaphores(list(self.sems.allocated().values()))
        nc.all_engine_barrier()

    _tile_mod.TileContext._drain_and_barrier = _split_drain_and_barrier


def _split_multi_waits(nc):
    import concourse.mybir as mybir
    n_split = 0
    for fn in nc.m.functions:
        for bb in fn.blocks:
            insts = list(bb.instructions)
            out = []
            changed = False
            for inst in insts:
                si = inst.sync_info
                if si is not None and len(si.on_wait) > 1:
                    waits = list(si.on_wait)
                    for w in waits[:-1]:
                        nop = mybir.InstNoOp(
                            name=f"{inst.name}-wsplit{n_split}",
                            engine=inst.engine, bass_nofuse=True)
                        nop.sync_info = mybir.SyncInfo(on_wait=[w], on_update=[])
                        out.append(nop)
                        n_split += 1
                    inst.sync_info = type(si)(
                        on_wait=[waits[-1]], on_update=list(si.on_update))
                    changed = True
                out.append(inst)
            if changed:
                bb.instructions = out
    return n_split


_apply_toolchain_patches()

import concourse.bass as bass  # noqa: E402
import concourse.tile as tile  # noqa: E402
from concourse import mybir  # noqa: E402
from concourse.bass_utils import run_bass_kernel_spmd  # noqa: E402

F32 = mybir.dt.float32
BF16 = mybir.dt.bfloat16

B, C, H, W = 8, 512, 64, 64
NH, HD = 8, 64
EPS = 1e-5
W65 = 65
SP65 = 64 * W65 + 4  # 4164
SP = 4096
N_CORES = 8

LAST_RESULTS = None


def _ap(t, offset, dims):
    return bass.AP(tensor=t.tensor, offset=t.offset + offset, ap=[t.ap[0]] + dims)


def _slice_part(t, p0, np_, offset, dims):
    # tile[p0:p0+np_] then rebuild free dims
    sub = t[p0:p0 + np_]
    return bass.AP(tensor=sub.tensor, offset=sub.offset + offset,
                   ap=[sub.ap[0]] + dims)


TAPS = [(1, 0), (1, 1), (1, 2), (0, 0), (0, 1), (0, 2), (2, 0), (2, 1), (2, 2)]


def conv3x3_chunk_gen(nc, psum, w_tiles, src_tiles, r0, n_ktiles, co=128):
    """3x3 conv, one 8-row chunk, accumulated into psum (co, 8*64).
    w_tiles[(tap_idx, kt)] = lhsT (K, co); src_tiles[kt] = w65 tile.
    dy=1 taps first (full coverage -> start=True clears the bank).
    Yields after every 4 matmuls so callers can interleave other PE work."""
    mms = []
    for dy, dx in TAPS:
        t_i = dy * 3 + dx  # host weight tap order is (ky, kx) row-major
        a = max(r0, 1 - dy)
        b = min(r0 + 8, 65 - dy)
        nr = b - a
        if nr <= 0:
            continue
        for kt in range(n_ktiles):
            mms.append((t_i, kt, a, nr, dy, dx))
    for j, (t_i, kt, a, nr, dy, dx) in enumerate(mms):
        src = src_tiles[kt]
        in_ap = _ap(src, (a + dy - 1) * W65 + dx, [[W65, nr], [1, 64]])
        nc.tensor.matmul(
            psum[0:co, (a - r0) * 64:(a - r0 + nr) * 64], w_tiles[(t_i, kt)],
            in_ap, start=(j == 0), stop=(j == len(mms) - 1),
            skip_group_check=True)
        if (j + 1) % 4 == 0:
            yield


def conv3x3_chunk(nc, psum, w_tiles, src_tiles, r0, n_ktiles, co=128):
    for _ in conv3x3_chunk_gen(nc, psum, w_tiles, src_tiles, r0, n_ktiles, co):
        pass


def build_program():
    nc = bass.Bass(trn_type="TRN2", num_devices=N_CORES)

    d_x65 = nc.dram_tensor("x65", [C, SP65], BF16, kind="ExternalInput")
    d_xT = nc.dram_tensor("xT", [C, SP], BF16, kind="ExternalInput")
    d_wqkT = nc.dram_tensor("wqkT", [C, 1024], BF16, kind="ExternalInput")
    d_wvT = nc.dram_tensor("wvT", [C, 512], BF16, kind="ExternalInput")
    d_bqk = nc.dram_tensor("bqk", [1, 1024], BF16, kind="ExternalInput")
    d_bv = nc.dram_tensor("bv", [1, 512], BF16, kind="ExternalInput")
    d_bw1 = nc.dram_tensor("bw1", [4, 128, 576], BF16, kind="ExternalInput")
    d_bb1 = nc.dram_tensor("bb1", [64, 1], F32, kind="ExternalInput")
    d_bw2 = nc.dram_tensor("bw2", [64, 4], BF16, kind="ExternalInput")
    d_bb2 = nc.dram_tensor("bb2", [4, 1], F32, kind="ExternalInput")
    d_ew1 = nc.dram_tensor("ew1", [4, 576], BF16, kind="ExternalInput")
    d_eb1 = nc.dram_tensor("eb1", [64, 1], F32, kind="ExternalInput")
    d_ew2 = nc.dram_tensor("ew2", [64, 8], BF16, kind="ExternalInput")
    d_eb2 = nc.dram_tensor("eb2", [8, 1], F32, kind="ExternalInput")
    d_gng = nc.dram_tensor("gng", [64, 1], F32, kind="ExternalInput")
    d_gnb = nc.dram_tensor("gnb", [64, 1], F32, kind="ExternalInput")
    d_gmat = nc.dram_tensor("gmat", [64, 8], F32, kind="ExternalInput")
    d_sel = nc.dram_tensor("sel", [8, 512], BF16, kind="ExternalInput")
    d_fw2 = nc.dram_tensor("fw2", [2, 4, 128, 4608], BF16, kind="ExternalInput")
    d_fb = nc.dram_tensor("fb", [C, 1], F32, kind="ExternalInput")
    d_bng = nc.dram_tensor("bng", [C, 1], F32, kind="ExternalInput")
    d_bnb = nc.dram_tensor("bnb", [C, 1], F32, kind="ExternalInput")
    d_y = nc.dram_tensor("y", [C, SP], F32, kind="ExternalOutput")

    AF = mybir.ActivationFunctionType

    with tile.TileContext(nc) as tc:
        with tc.tile_pool(name="glob", bufs=1) as glob, \
             tc.tile_pool(name="psA", bufs=3, space="PSUM") as psA, \
             tc.tile_pool(name="psB", bufs=3, space="PSUM") as psB, \
             tc.tile_pool(name="psF", bufs=2, space="PSUM") as psF, \
             tc.tile_pool(name="dram", bufs=1, space="DRAM") as dram:

            x65 = [glob.tile([128, SP65], BF16, name=f"x65_{k}") for k in range(4)]
            ones1 = glob.tile([1, 128], BF16)
            # split loads across both HWDGE queues AND by row halves: the
            # first box-conv chunks only read rows 0..9, so they can start
            # once the (much smaller) first halves land
            HALF = 32 * W65
            for k in range(4):
                eng = nc.sync if k % 2 == 0 else nc.scalar
                eng.dma_start(out=x65[k][:, 0:HALF],
                              in_=d_x65[k * 128:(k + 1) * 128, 0:HALF])
            for k in range(4):
                eng = nc.sync if k % 2 == 0 else nc.scalar
                eng.dma_start(out=x65[k][:, HALF:SP65],
                              in_=d_x65[k * 128:(k + 1) * 128, HALF:SP65])
            nc.vector.memset(ones1, 1.0)
            agg65 = [glob.tile([128, SP65], BF16, name=f"agg65_{k}")
                     for k in range(4)]
            for k in range(4):
                nc.vector.memset(agg65[k], 0.0)
            xT = [glob.tile([128, SP], BF16, name=f"xT_{k}") for k in range(4)]
            sel8 = glob.tile([8, 512], BF16)
            nc.sync.dma_start(out=sel8, in_=d_sel[:])
            edge_fl = glob.tile([8, SP], BF16)
            fb = glob.tile([128, 4], F32)
            nc.sync.dma_start(
                out=fb, in_=d_fb[:].rearrange("(a b) c -> b (a c)", a=4))
            bng = glob.tile([128, 4], F32)
            nc.sync.dma_start(
                out=bng, in_=d_bng[:].rearrange("(a b) c -> b (a c)", a=4))
            bnb = glob.tile([128, 4], F32)
            nc.sync.dma_start(
                out=bnb, in_=d_bnb[:].rearrange("(a b) c -> b (a c)", a=4))
            stats_l = glob.tile([128, 8], F32)
            d_ytx = dram.tile([C, SP], BF16)  # fusion x-half partial bounce
            d_aggB = [dram.tile([64, 8192], BF16, name=f"aggB_{g}")
                      for g in range(4)]  # agg bounce, (d, c_loc*64+w)

            # fusion conv weight streaming pool (lives across ph2 + ph3)
            with tc.tile_pool(name="fwp", bufs=1) as fwp:

                def load_fw(half, ct):
                    """One packed DMA for all 36 lhsT tiles of a fusion-conv
                    (half, ct) group; returns the (tap, kt)->lhsT AP dict."""
                    fw_all = fwp.tile([128, 4608], BF16, tag="fwall", bufs=2,
                                      name=f"fwall_{half}_{ct}")
                    nc.sync.dma_start(out=fw_all, in_=d_fw2[half, ct])
                    return {(t_i, kt): fw_all[:, (t_i * 4 + kt) * 128:
                                              (t_i * 4 + kt + 1) * 128]
                            for t_i in range(9) for kt in range(4)}

                def emit_xhalf_ct(ct):
                    """Fusion-conv x-half (input channels 0..511) for output
                    channel tile ct. PE filler work: depends only on x65 +
                    weights. Result (incl. fusion bias) -> d_ytx in bf16.
                    Generator: yields every 4 matmuls for PE interleaving."""
                    fw = load_fw(0, ct)
                    for ch in range(8):
                        pf = psF.tile([128, 512], F32, tag="fx",
                                      name=f"pfx_{ct}_{ch}")
                        yield from conv3x3_chunk_gen(nc, pf, fw, x65,
                                                     ch * 8, 4)
                        st = fwp.tile([128, 512], BF16, tag="ytxs", bufs=4,
                                      name=f"ytxs_{ct}_{ch}")
                        nc.vector.tensor_scalar_add(out=st, in0=pf,
                                                    scalar1=fb[:, ct:ct + 1])
                        nc.sync.dma_start(
                            out=d_ytx[ct * 128:(ct + 1) * 128,
                                      ch * 512:(ch + 1) * 512],
                            in_=st)

                class _Fillers:
                    """Doles out x-half fusion matmul quanta (4 MMs each) as
                    PE filler, consuming ct groups 0..3 in order."""
                    def __init__(self):
                        self.next_ct = 0
                        self.cur = None

                    def step(self, n=1):
                        for _ in range(n):
                            while True:
                                if self.cur is None:
                                    if self.next_ct >= 4:
                                        return
                                    self.cur = emit_xhalf_ct(self.next_ct)
                                    self.next_ct += 1
                                try:
                                    next(self.cur)
                                    break
                                except StopIteration:
                                    self.cur = None

                    def drain_all(self):
                        while self.cur is not None or self.next_ct < 4:
                            if self.cur is None:
                                self.cur = emit_xhalf_ct(self.next_ct)
                                self.next_ct += 1
                            for _ in self.cur:
                                pass
                            self.cur = None

                fillers = _Fillers()

                # ---------- Phase 1: box_net + edge_net ----------
                if True:
                    with tc.tile_pool(name="p1", bufs=1) as p1:
                        bw1_all = [p1.tile([128, 576], BF16, name=f"bw1a_{kt}")
                                   for kt in range(4)]
                        for kt in range(4):
                            nc.sync.dma_start(out=bw1_all[kt], in_=d_bw1[kt])
                        bw1 = {(t_i, kt):
                               bw1_all[kt][:, t_i * 64:(t_i + 1) * 64]
                               for t_i in range(9) for kt in range(4)}
                        bb1 = p1.tile([64, 1], F32)
                        nc.sync.dma_start(out=bb1, in_=d_bb1[:])
                        bw2 = p1.tile([64, 4], BF16)
                        nc.sync.dma_start(out=bw2, in_=d_bw2[:])
                        bb2 = p1.tile([4, 1], F32)
                        nc.sync.dma_start(out=bb2, in_=d_bb2[:])
                        ew1_all = p1.tile([4, 576], BF16, name="ew1a")
                        nc.sync.dma_start(out=ew1_all, in_=d_ew1[:])
                        ew1 = {(t_i, 0): ew1_all[:, t_i * 64:(t_i + 1) * 64]
                               for t_i in range(9)}
                        eb1 = p1.tile([64, 1], F32)
                        nc.sync.dma_start(out=eb1, in_=d_eb1[:])
                        ew2 = p1.tile([64, 8], BF16)
                        nc.sync.dma_start(out=ew2, in_=d_ew2[:])
                        eb2 = p1.tile([8, 1], F32)
                        nc.sync.dma_start(out=eb2, in_=d_eb2[:])
                        gng = p1.tile([64, 1], F32)
                        nc.sync.dma_start(out=gng, in_=d_gng[:])
                        gnb = p1.tile([64, 1], F32)
                        nc.sync.dma_start(out=gnb, in_=d_gnb[:])
                        gmat = p1.tile([64, 8], F32)
                        nc.sync.dma_start(out=gmat, in_=d_gmat[:])
                        # xT loads deferred here: box weights reach SBUF
                        # first so the box conv starts ~15us earlier
                        for k in range(4):
                            nc.sync.dma_start(
                                out=xT[k], in_=d_xT[k * 128:(k + 1) * 128, :])

                        box1 = p1.tile([64, SP65], BF16)
                        nc.vector.memset(box1, 0.0)
                        for ch in range(8):
                            pb = psA.tile([128, 512], F32, tag="conv",
                                          name=f"pb_{ch}")
                            conv3x3_chunk(nc, pb, bw1, x65, ch * 8, 4, co=64)
                            nc.scalar.activation(
                                out=_slice_part(box1, 0, 64, ch * 8 * W65 + 1,
                                                [[W65, 8], [1, 64]]),
                                in_=pb[0:64, :], func=AF.Gelu, bias=bb1,
                                scale=1.0)

                        boxes = p1.tile([4, SP65], BF16)
                        nc.vector.memset(boxes, 0.0)
                        for ch in range(8):
                            pb2 = psA.tile([128, 512], F32, tag="conv",
                                           name=f"pb2_{ch}")
                            nc.tensor.matmul(
                                pb2[0:4, :], bw2,
                                _ap(box1, ch * 8 * W65 + 1, [[W65, 8], [1, 64]]),
                                start=True, stop=True)
                            nc.scalar.activation(
                                out=_slice_part(boxes, 0, 4, ch * 8 * W65 + 1,
                                                [[W65, 8], [1, 64]]),
                                in_=pb2[0:4, :], func=AF.Sigmoid, bias=bb2,
                                scale=1.0)
                            fillers.step(1)

                        e1 = p1.tile([64, SP], F32)
                        for ch in range(8):
                            pe = psA.tile([128, 512], F32, tag="conv",
                                          name=f"pe_{ch}")
                            conv3x3_chunk(nc, pe, ew1, [boxes], ch * 8, 1, co=64)
                            nc.scalar.activation(
                                out=e1[:, ch * 512:(ch + 1) * 512],
                                in_=pe[0:64, :],
                                func=AF.Identity, bias=eb1, scale=1.0)
                            fillers.step(1)

                        stats = p1.tile([64, 8, 6], F32)
                        for j in range(8):
                            nc.vector.bn_stats(out=stats[:, j, :],
                                               in_=e1[:, j * 512:(j + 1) * 512])
                        mv = p1.tile([64, 2], F32)
                        nc.vector.bn_aggr(out=mv, in_=stats)
                        ex2 = p1.tile([64, 2], F32)
                        nc.vector.tensor_copy(out=ex2[:, 0:1], in_=mv[:, 0:1])
                        nc.vector.tensor_mul(out=ex2[:, 1:2], in0=mv[:, 0:1],
                                             in1=mv[:, 0:1])
                        nc.vector.tensor_add(out=ex2[:, 1:2], in0=ex2[:, 1:2],
                                             in1=mv[:, 1:2])
                        gs_ps = psB.tile([8, 2], F32, tag="att", name="gs_ps")
                        nc.tensor.matmul(gs_ps, gmat, ex2, start=True, stop=True)
                        gs = p1.tile([8, 2], F32)
                        nc.scalar.activation(out=gs, in_=gs_ps, func=AF.Copy,
                                             bias=0.0, scale=1.0 / 8.0)
                        gvar = p1.tile([8, 1], F32)
                        eps8 = p1.tile([8, 1], F32)
                        nc.vector.memset(eps8, float(EPS))
                        nc.vector.tensor_mul(out=gvar, in0=gs[:, 0:1],
                                             in1=gs[:, 0:1])
                        nc.vector.tensor_sub(out=gvar, in0=gs[:, 1:2], in1=gvar)
                        nc.scalar.activation(out=gvar, in_=gvar, func=AF.Sqrt,
                                             bias=eps8, scale=1.0)
                        nc.vector.reciprocal(out=gvar, in_=gvar)
                        gmr = p1.tile([8, 2], F32)
                        nc.vector.tensor_copy(out=gmr[:, 0:1], in_=gs[:, 0:1])
                        nc.vector.tensor_copy(out=gmr[:, 1:2], in_=gvar)
                        cmr = p1.tile([64, 2], F32)
                        src_bc = bass.AP(tensor=gmr.tensor, offset=gmr.offset,
                                         ap=[[gmr.ap[0][0], 8], [0, 8], [1, 2]])
                        nc.sync.dma_start(out=cmr, in_=src_bc)
                        gsc = p1.tile([64, 1], F32)
                        nc.vector.tensor_mul(out=gsc, in0=cmr[:, 1:2], in1=gng)
                        gsh = p1.tile([64, 1], F32)
                        nc.vector.tensor_mul(out=gsh, in0=cmr[:, 0:1], in1=gsc)
                        nc.vector.tensor_sub(out=gsh, in0=gnb, in1=gsh)
                        fillers.step(6)
                        e1g = p1.tile([64, SP], BF16)
                        nc.scalar.activation(out=e1g, in_=e1, func=AF.Gelu,
                                             bias=gsh, scale=gsc)
                        for ch in range(8):
                            pe2 = psA.tile([128, 512], F32, tag="conv",
                                           name=f"pe2_{ch}")
                            nc.tensor.matmul(pe2[0:8, :], ew2,
                                             e1g[:, ch * 512:(ch + 1) * 512],
                                             start=True, stop=True)
                            nc.scalar.activation(
                                out=edge_fl[:, ch * 512:(ch + 1) * 512],
                                in_=pe2[0:8, :], func=AF.Identity, bias=eb2,
                                scale=1.0)
                            fillers.step(1)

                # ---------- Phase 2: qkv conv + attention ----------
                with tc.tile_pool(name="p2", bufs=1) as p2:
                    wqkT = [p2.tile([128, 1024], BF16, name=f"wqkT_{kt}")
                            for kt in range(4)]
                    wvT = [p2.tile([128, 512], BF16, name=f"wvT_{kt}")
                           for kt in range(4)]
                    for kt in range(4):
                        nc.sync.dma_start(out=wqkT[kt],
                                          in_=d_wqkT[kt * 128:(kt + 1) * 128, :])
                        nc.sync.dma_start(out=wvT[kt],
                                          in_=d_wvT[kt * 128:(kt + 1) * 128, :])
                    bqk = p2.tile([1, 1024], BF16)
                    nc.sync.dma_start(out=bqk, in_=d_bqk[:])
                    bv = p2.tile([1, 512], BF16)
                    nc.sync.dma_start(out=bv, in_=d_bv[:])
                    # partition-broadcast copies: bias rides the psum->SBUF
                    # vector adds instead of K=1 matmuls (those stream N
                    # cycles at ~zero PE-array duty and cool the HAM gate)
                    bqk64 = p2.tile([64, 1024], BF16)
                    nc.sync.dma_start(
                        out=bqk64,
                        in_=bass.AP(tensor=bqk.tensor, offset=bqk.offset,
                                    ap=[bqk.ap[0], [0, 64], [1, 1024]]))
                    bv64 = p2.tile([64, 512], BF16)
                    nc.sync.dma_start(
                        out=bv64,
                        in_=bass.AP(tensor=bv.tensor, offset=bv.offset,
                                    ap=[bv.ap[0], [0, 64], [1, 512]]))

                    for g in range(4):
                        # qka and va share one slot (tag "qv"): qka dies
                        # after the score matmuls, va is built right after.
                        qka = p2.tile([64, 64, 256], BF16, tag="qv",
                                      name=f"qka_{g}")
                        # qk conv: x^T stationary, column pairs
                        for sp in range(32):
                            pq = psA.tile([128, 256], F32, tag="conv",
                                          name=f"pq_{g}_{sp}")
                            for kt in range(4):
                                nc.tensor.matmul(
                                    pq, xT[kt][:, sp * 128:(sp + 1) * 128],
                                    wqkT[kt][:, g * 256:(g + 1) * 256],
                                    start=(kt == 0), stop=(kt == 3))
                            for j in range(2):
                                nc.vector.tensor_add(
                                    out=_ap(qka, (2 * sp + j) * 256, [[1, 256]]),
                                    in0=pq[64 * j:64 * (j + 1), :],
                                    in1=bqk64[:, g * 256:(g + 1) * 256])
                            # light filler keeps array duty up through the
                            # LDW-bound qk stream
                            if sp % 4 == 3:
                                fillers.step(1)
                        # logits + exp, per head
                        Pn = [None, None]
                        for hh in range(2):
                            h = 2 * g + hh
                            qoff, koff = hh * 128, hh * 128 + 64
                            Pn[hh] = p2.tile([64, SP], BF16, tag="Pn",
                                             bufs=2, name=f"Pn_{g}_{hh}")
                            for ib in range(8):
                                sp_ = psB.tile([64, 512], F32, tag="att",
                                               name=f"sp_{g}_{hh}_{ib}")
                                for ii in range(8):
                                    i = ib * 8 + ii
                                    nc.tensor.matmul(
                                        sp_[:, ii * 64:(ii + 1) * 64],
                                        _ap(qka, koff + i, [[256, 64]]),
                                        _ap(qka, qoff + i, [[256, 64]]),
                                        start=(ii == 0), stop=False,
                                        skip_group_check=True)
                                nc.tensor.matmul(
                                    sp_, sel8[:, h * 64:(h + 1) * 64],
                                    edge_fl[:, ib * 512:(ib + 1) * 512],
                                    start=False, stop=True,
                                    skip_group_check=True)
                                nc.scalar.activation(
                                    out=_ap(Pn[hh], ib * 512, [[1, 512]]),
                                    in_=sp_, func=AF.Exp)
                        # v conv: x stationary, one row per matmul (M=64;
                        # walrus allows only one free dim on the stationary
                        # operand), with x-half fusion matmuls interleaved
                        # as PE filler so the HAM clock-gate stays warm
                        va = p2.tile([64, 64, 128], BF16, tag="qv",
                                     name=f"va_{g}")
                        for sp in range(32):
                            for j in range(2):
                                d0 = 2 * sp + j
                                pv = psA.tile([64, 128], F32, tag="conv",
                                              name=f"pv_{g}_{d0}")
                                for kt in range(4):
                                    lhs = _ap(x65[kt], d0 * W65 + 1, [[1, 64]])
                                    nc.tensor.matmul(
                                        pv, lhs,
                                        wvT[kt][:, g * 128:(g + 1) * 128],
                                        start=(kt == 0), stop=(kt == 3))
                                nc.vector.tensor_add(
                                    out=_ap(va, d0 * 128, [[1, 128]]), in0=pv,
                                    in1=bv64[:, g * 128:(g + 1) * 128])
                                # interleave x-half fusion matmuls as PE
                                # filler (HAM warmth); hold some back on the
                                # last group to cover the phase-3 boundary
                                if g < 2 or j == 0:
                                    fillers.step(1)
                        # softmax normalize + AV per head
                        for hh in range(2):
                            h = 2 * g + hh
                            Px = Pn[hh]
                            D = p2.tile([64, 64], F32, tag=f"D{hh}",
                                        name=f"D_{g}_{hh}")
                            Pv = bass.AP(tensor=Px.tensor, offset=Px.offset,
                                         ap=[Px.ap[0], [1, 64], [64, 64]])
                            nc.vector.reduce_sum(out=D, in_=Pv,
                                                 axis=mybir.AxisListType.X)
                            nc.vector.reciprocal(out=D, in_=D)
                            Rb = bass.AP(tensor=D.tensor, offset=D.offset,
                                         ap=[D.ap[0], [0, 64], [1, 64]])
                            nc.vector.tensor_mul(out=Px, in0=Px, in1=Rb)
                            blk = p2.tile([64, SP], BF16, tag="blk",
                                          name=f"blk_{g}_{hh}")
                            for ib in range(8):
                                ap2 = psB.tile([64, 512], F32, tag="att",
                                               name=f"ap2_{g}_{hh}_{ib}")
                                for ii in range(8):
                                    i = ib * 8 + ii
                                    nc.tensor.matmul(
                                        ap2[:, ii * 64:(ii + 1) * 64],
                                        _ap(va, hh * 64 + i, [[128, 64]]),
                                        Px[:, i * 64:(i + 1) * 64],
                                        start=(ii == 0), stop=(ii == 7),
                                        skip_group_check=True)
                                nc.vector.tensor_copy(
                                    out=blk[:, ib * 512:(ib + 1) * 512],
                                    in_=ap2)
                            # contiguous write: (d, c_loc*64+w) block layout
                            nc.sync.dma_start(
                                out=_ap(d_aggB[g], hh * 4096, [[1, 4096]]),
                                in_=blk)
                            # agg -> conv world: strided read-back per head
                            # (starts during the other head's AV)
                            nc.sync.dma_start(
                                out=_slice_part(agg65[g], hh * 64, 64, 1,
                                                [[W65, 64], [1, 64]]),
                                in_=bass.AP(
                                    tensor=d_aggB[g].tensor,
                                    offset=d_aggB[g].offset + hh * 4096,
                                    ap=[[64, 64], [8192, 64], [1, 64]]))
                        # after the last group, drain all remaining x-half
                        # filler work to cover the agg read-back latency
                        if g == 3:
                            fillers.drain_all()

                # ================= Phase 3: agg-half + BN =================
                with tc.tile_pool(name="p3", bufs=1) as p3:
                    ypre = [p3.tile([128, SP], BF16, name=f"ypre_{k}")
                            for k in range(4)]
                    eps128 = p3.tile([128, 1], F32)
                    nc.vector.memset(eps128, float(EPS))
                    for ct in range(4):
                        fw = load_fw(1, ct)
                        st6 = p3.tile([128, 8, 6], F32, tag="st6",
                                      name=f"st6_{ct}")
                        for ch in range(8):
                            ytr = fwp.tile([128, 512], BF16, tag="ytxr",
                                           bufs=3, name=f"ytxr_{ct}_{ch}")
                            nc.sync.dma_start(
                                out=ytr,
                                in_=d_ytx[ct * 128:(ct + 1) * 128,
                                          ch * 512:(ch + 1) * 512])
                            pf = (psA if ch % 2 == 0 else psF).tile(
                                [128, 512], F32,
                                tag=("conv" if ch % 2 == 0 else "fx"),
                                name=f"pfa_{ct}_{ch}")
                            conv3x3_chunk(nc, pf, fw, agg65, ch * 8, 4)
                            yslice = ypre[ct][:, ch * 512:(ch + 1) * 512]
                            nc.vector.tensor_add(out=yslice, in0=pf, in1=ytr)
                            nc.vector.bn_stats(out=st6[:, ch, :], in_=yslice)
                        mv4 = p3.tile([128, 2], F32, tag="mv4", name=f"mv4_{ct}")
                        nc.vector.bn_aggr(out=mv4, in_=st6)
                        nc.scalar.activation(out=stats_l[:, 2 * ct:2 * ct + 1],
                                             in_=mv4[:, 0:1], func=AF.Copy,
                                             bias=0.0, scale=float(SP))
                        sq = p3.tile([128, 1], F32, tag="sq", name=f"sq_{ct}")
                        nc.vector.tensor_mul(out=sq, in0=mv4[:, 0:1],
                                             in1=mv4[:, 0:1])
                        nc.vector.tensor_add(out=sq, in0=sq, in1=mv4[:, 1:2])
                        nc.scalar.activation(
                            out=stats_l[:, 2 * ct + 1:2 * ct + 2],
                            in_=sq, func=AF.Copy, bias=0.0, scale=float(SP))

                    NTOT = float(B * SP)

                    def bn_apply(cts, rstats, ncols):
                        """AllReduced (sum, sumsq) cols -> sc/sh -> fused
                        Silu+store for the given ct list."""
                        mean = p3.tile([128, ncols], F32, tag=f"mn{ncols}")
                        nc.scalar.activation(out=mean,
                                             in_=_ap(rstats, 0, [[2, ncols]]),
                                             func=AF.Copy, bias=0.0,
                                             scale=1.0 / NTOT)
                        e2 = p3.tile([128, ncols], F32, tag=f"e2{ncols}")
                        nc.scalar.activation(out=e2,
                                             in_=_ap(rstats, 1, [[2, ncols]]),
                                             func=AF.Copy, bias=0.0,
                                             scale=1.0 / NTOT)
                        var = p3.tile([128, ncols], F32, tag=f"vr{ncols}")
                        nc.vector.tensor_mul(out=var, in0=mean, in1=mean)
                        nc.vector.tensor_sub(out=var, in0=e2, in1=var)
                        nc.scalar.activation(out=var, in_=var, func=AF.Sqrt,
                                             bias=eps128, scale=1.0)
                        nc.vector.reciprocal(out=var, in_=var)
                        sc = p3.tile([128, ncols], F32, tag=f"sc{ncols}")
                        nc.vector.tensor_mul(out=sc, in0=var,
                                             in1=bng[:, cts[0]:cts[0] + ncols])
                        sh = p3.tile([128, ncols], F32, tag=f"sh{ncols}")
                        nc.vector.tensor_mul(out=sh, in0=mean, in1=sc)
                        nc.vector.tensor_sub(out=sh,
                                             in0=bnb[:, cts[0]:cts[0] + ncols],
                                             in1=sh)
                        for ci, ct in enumerate(cts):
                            for half in range(2):
                                sg = p3.tile([128, SP // 2], F32, tag="sg",
                                             bufs=3, name=f"sg_{ct}_{half}")
                                nc.scalar.activation(
                                    out=sg,
                                    in_=ypre[ct][:, half * 2048:
                                                 (half + 1) * 2048],
                                    func=AF.Silu, bias=sh[:, ci:ci + 1],
                                    scale=sc[:, ci:ci + 1])
                                nc.sync.dma_start(
                                    out=d_y[ct * 128:(ct + 1) * 128,
                                            half * 2048:(half + 1) * 2048],
                                    in_=sg)

                    # split AllReduce: cts 0-2 fire after ct2's stats and
                    # hide (CC + BN math + Silu + store) under ct3's conv;
                    # only ct3's small CC is exposed at the tail.
                    cc_in_a = dram.tile([128, 6], F32)
                    cc_out_a = dram.tile([128, 6], F32)
                    nc.sync.dma_start(out=cc_in_a, in_=stats_l[:, 0:6])
                    nc.gpsimd.collective_compute(
                        "AllReduce", mybir.AluOpType.add,
                        replica_groups=[list(range(N_CORES))],
                        ins=[cc_in_a.opt()], outs=[cc_out_a.opt()])
                    rstats_a = p3.tile([128, 6], F32)
                    nc.sync.dma_start(out=rstats_a, in_=cc_out_a)
                    bn_apply([0, 1, 2], rstats_a, 3)

                    cc_in_b = dram.tile([128, 2], F32)
                    cc_out_b = dram.tile([128, 2], F32)
                    nc.sync.dma_start(out=cc_in_b, in_=stats_l[:, 6:8])
                    nc.gpsimd.collective_compute(
                        "AllReduce", mybir.AluOpType.add,
                        replica_groups=[list(range(N_CORES))],
                        ins=[cc_in_b.opt()], outs=[cc_out_b.opt()])
                    rstats_b = p3.tile([128, 2], F32)
                    nc.sync.dma_start(out=rstats_b, in_=cc_out_b)
                    bn_apply([3], rstats_b, 1)

    _split_multi_waits(nc)
    return nc


_PROGRAM = None


def _get_program():
    global _PROGRAM
    if _PROGRAM is None:
        _PROGRAM = build_program()
    return _PROGRAM


def _bf16(a):
    return np.ascontiguousarray(np.asarray(a, np.float32).astype(ml_dtypes.bfloat16))


def _f32(a):
    return np.ascontiguousarray(np.asarray(a, np.float32))


def kernel(x, box_w1, box_b1, box_w2, box_b2, edge_w1, edge_b1, gn_g, gn_b,
           edge_w2, edge_b2, qkv_w, qkv_b, fus_w, fus_b, bn_g, bn_b,
           trace=False):
    global LAST_RESULTS
    x = np.asarray(x, np.float32)
    scale = float(HD) ** -0.5

    qkv_w2 = np.asarray(qkv_w, np.float32).reshape(3 * C, C)
    qkv_b2 = np.asarray(qkv_b, np.float32).copy()
    wq = qkv_w2[0:C] * scale
    bq = qkv_b2[0:C] * scale
    wk, bk = qkv_w2[C:2 * C], qkv_b2[C:2 * C]
    wv, bv_ = qkv_w2[2 * C:], qkv_b2[2 * C:]
    wqk = np.empty((1024, C), np.float32)
    bqk = np.empty(1024, np.float32)
    for h in range(NH):
        wqk[h * 128:h * 128 + 64] = wq[h * 64:(h + 1) * 64]
        wqk[h * 128 + 64:(h + 1) * 128] = wk[h * 64:(h + 1) * 64]
        bqk[h * 128:h * 128 + 64] = bq[h * 64:(h + 1) * 64]
        bqk[h * 128 + 64:(h + 1) * 128] = bk[h * 64:(h + 1) * 64]

    bw1T = np.asarray(box_w1, np.float32).transpose(2, 3, 1, 0).reshape(9, C, 64)
    # packed box weights: bw1p[kt][p, t*64+co] = bw1T[t, kt*128+p, co]
    bw1p = np.ascontiguousarray(
        bw1T.reshape(9, 4, 128, 64).transpose(1, 2, 0, 3).reshape(4, 128, 576))
    ew1T = np.asarray(edge_w1, np.float32).transpose(2, 3, 1, 0).reshape(9, 4, 64)
    ew1p = np.ascontiguousarray(
        ew1T.transpose(1, 0, 2).reshape(4, 576))
    fwT = np.asarray(fus_w, np.float32).transpose(2, 3, 1, 0).reshape(9, 1024, C)
    fwT_t = fwT.reshape(9, 8, 128, 4, 128).transpose(0, 1, 3, 2, 4)
    # packed fusion weights: fw2[half, ct, p, (t*4+ktl)*128+co]
    fw2 = np.ascontiguousarray(
        fwT_t.reshape(9, 2, 4, 4, 128, 128).transpose(1, 3, 4, 0, 2, 5)
        .reshape(2, 4, 128, 4608))

    gmat = np.zeros((64, 8), np.float32)
    for g in range(8):
        gmat[g * 8:(g + 1) * 8, g] = 1.0

    shared = {
        "wqkT": _bf16(wqk.T), "wvT": _bf16(wv.T),
        "bqk": _bf16(bqk[None, :]), "bv": _bf16(bv_[None, :]),
        "bw1": _bf16(bw1p), "bb1": _f32(np.asarray(box_b1).reshape(64, 1)),
        "bw2": _bf16(np.asarray(box_w2, np.float32).reshape(4, 64).T),
        "bb2": _f32(np.asarray(box_b2).reshape(4, 1)),
        "ew1": _bf16(ew1p), "eb1": _f32(np.asarray(edge_b1).reshape(64, 1)),
        "ew2": _bf16(np.asarray(edge_w2, np.float32).reshape(8, 64).T),
        "eb2": _f32(np.asarray(edge_b2).reshape(8, 1)),
        "gng": _f32(np.asarray(gn_g).reshape(64, 1)),
        "gnb": _f32(np.asarray(gn_b).reshape(64, 1)),
        "gmat": gmat,
        "sel": _bf16(np.kron(np.eye(8, dtype=np.float32),
                             np.ones((1, 64), np.float32))),
        "fw2": _bf16(fw2),
        "fb": _f32(np.asarray(fus_b).reshape(C, 1)),
        "bng": _f32(np.asarray(bn_g).reshape(C, 1)),
        "bnb": _f32(np.asarray(bn_b).reshape(C, 1)),
    }

    in_maps = []
    for b in range(B):
        xb = x[b]
        x65h = np.zeros((C, 64, W65), np.float32)
        x65h[:, :, 1:] = xb
        x65h = np.concatenate(
            [x65h.reshape(C, 4160), np.zeros((C, 4), np.float32)], axis=1)
        xTh = np.ascontiguousarray(xb.transpose(0, 2, 1)).reshape(C, SP)
        m = dict(shared)
        m["x65"] = _bf16(x65h)
        m["xT"] = _bf16(xTh)
        in_maps.append(m)

    nc = _get_program()
    res = run_bass_kernel_spmd(nc, in_maps, core_ids=list(range(N_CORES)),
                               trace=trace)
    LAST_RESULTS = res
    out = np.empty((B, C, H, W), np.float32)
    for b in range(B):
        out[b] = res.results[b]["y"].reshape(C, H, W)
    return out
